# revision 103
# baseline (speedup 1.0000x reference)
"""AttentionPairBias Trainium2 kernel (8 NeuronCores, SPMD over query rows).

Sharding: the 768 query rows are split 96-per-core. Each core computes the
full output rows for its query slice; the host concatenates.

Device-side math (per core). Both layer norms fold on the host
(elementwise / per-row reductions only, no contractions):
  an  = LN(a)                        shipped pre-transposed, bf16
  z'  = z * rstd(z)                  shipped fp8 (half the bytes of z+z^2)
  Wz'' = w*Wz - colsum(w*Wz)/CZ      so  pair_bias = z' @ Wz''  exactly
  (the LN-beta per-head constant cancels in the softmax and is dropped)

z pipeline: 36 chunks of [CZ, 16q x 128k].  Each chunk is 4 col-tiled fp8
matmuls (positions (0,32b) run concurrently on the PE); chunk PAIRS share
one psA bank via two half-stationaries [wza|0] / [0|wza] (even chunk
writes quadrant rows 0:16, odd accumulates rows 16:32), which halves both
the psA->sbuf copies and the PE transposes.  Transposed psT tiles land in
the PSUM-native (kt, qg, q'=s*4+b, h) order and are staged to SBUF by a
single contiguous vector op per pair; the host permutes every q-indexed
tensor by the same 4x4-transposition involution (PERM96) and de-permutes
output rows, so no strided writes exist anywhere in the bias path.

Attention: per (head, 3-kt group), qk matmuls accumulate into a scores
bank, the vector engine adds the staged bias (strided bf16 read), one Exp
per group, pv matmuls col-tiled per head quadrant.  kg0 scores+exp thread
into the back half of the z loop (p_t parks in SBUF); only the kg1 sweep,
pv, and per-cn tails (reciprocal via ones-broadcast matmul, gating from
the Exp-table sigmoid, Wo projection) run after the stream ends.

Scheduling notes:
 - all DMA queues matter: z' streams on the sync HWDGE queue as 9x 8KB-
   per-partition descriptors; the weight blob ships as 2048-col pieces on
   the scalar queue because the fabric round-robins queues per descriptor
   (oversized const descriptors starve the z stream, undersized blob
   pieces starve the blob)
 - z groups are prefetched 16 chunks deep; blob pieces trickle between
   them; phase-A projection pieces start at chunk 10 (qg first, then k
   halves kg0-first, then v) and all 16 kg0 score units thread in from
   chunk 20, so nothing blocks the in-order PE queue
 - gpsimd only memsets (it cannot touch PSUM and its DMAs go through the
   slow software DGE); the scalar table only ever holds Exp (one load)
"""

import os
import sys
import numpy as np

sys.path.insert(0, "/opt/trn_rl_repo")
os.environ.setdefault("MYCRO_LOCAL_CACHE", "1")

from ml_dtypes import bfloat16, float8_e4m3

# ---- problem constants (hardcoded per the harness contract) ----
B, N, C, CZ, H, CH = 1, 768, 384, 128, 16, 24
NCORES = 8
NQ = N // NCORES          # 96 query rows per core
CHP = 32                  # padded per-head width
HP = H * CHP              # 512 padded hc
EPS = 1e-5
INF = 1e9
KT = N // 128             # 6 key tiles
QG = 16                   # query rows per z-chunk
NQG = NQ // QG            # 6 query groups
NCHUNK = KT * NQG         # 36 chunks, key-tile major
NBLK = 4                  # 512-wide output blocks per chunk
DG = 4                    # chunks per z DMA (8KB/partition descriptors)
KG = 3                    # key tiles per scores group
# within-16 query transposition (involution): device col p <-> query 4(p%4)+p//4
PERM96 = np.arange(NQ).reshape(NQG, 4, 4).transpose(0, 2, 1).reshape(NQ)

# bf16 constant blob layout (columns, all [128, x]).  Order matters: the
# first section ships in an early DMA (k/q/g projections unblock first).
_BLOB = {}
_off = 0
for _nm, _w in [("anTq", 3 * NQ), ("wq", 4 * HP), ("wg", 4 * HP),
                ("anT", 3 * N), ("wk", 4 * HP),
                ("wv", 3 * C), ("wo", 4 * C)]:
    _BLOB[_nm] = (_off, _w)
    _off += _w
BLOBW = _off

_CACHE = {}


def _build_program():
    from contextlib import ExitStack
    import concourse.bass as bass
    import concourse.tile as tile
    from concourse import bacc, mybir

    f32 = mybir.dt.float32
    b16 = mybir.dt.bfloat16
    f8 = mybir.dt.float8e4
    AF = mybir.ActivationFunctionType
    OP = mybir.AluOpType

    nc = bacc.Bacc("TRN2", target_bir_lowering=False, debug=False)

    # ---- DRAM I/O ----
    # z' fp8 chunks: per chunk [CZ, 2048] (16 q x 128 k, k minor)
    zt_d = nc.dram_tensor("zt", [CZ, NCHUNK * 2048], f8, kind="ExternalInput")
    # two half-stationaries: [wza|0] and [0|wza] — chunk pairs share one
    # psA bank (even chunk writes quadrant rows 0:16, odd accumulates 16:32)
    wza_d = nc.dram_tensor("wza", [CZ, 2, 32], f8, kind="ExternalInput")
    id_d = nc.dram_tensor("ident", [128, 128], b16, kind="ExternalInput")
    blob_d = nc.dram_tensor("blob", [128, BLOBW], b16, kind="ExternalInput")
    # f32 sidecar: bo row | mask cols | -bg activation-bias cols
    bob_d = nc.dram_tensor("bob", [128, C + KT + 4], f32, kind="ExternalInput")
    out_d = nc.dram_tensor("out", [NQ, C], f32, kind="ExternalOutput")

    with tile.TileContext(nc) as tc, ExitStack() as ctx:
        const = ctx.enter_context(tc.tile_pool(name="const", bufs=1))

        # ------------- constant loads.  The blob ships as 2048-col pieces
        # so its DMA descriptors match the z stream's (the fabric round-
        # robins by descriptor; oversized const descriptors starve z) ------
        wzaug = const.tile([CZ, 2, 32], f8)
        nc.scalar.dma_start(wzaug, wza_d[:, :, :])
        sb_id = const.tile([128, 128], b16)
        nc.scalar.dma_start(sb_id, id_d[:, :])
        bob = const.tile([128, C + KT + 4], f32)
        nc.scalar.dma_start(bob, bob_d[:, :])
        blob = const.tile([128, BLOBW], b16)
        BP = 2048

        def _blob_dma(i):
            lo, hi = BP * i, min(BP * (i + 1), BLOBW)
            nc.scalar.dma_start(blob[:, lo:hi], blob_d[:, lo:hi])



        def _bv(nm, c):
            o, w = _BLOB[nm]
            return blob[:, o:o + w].rearrange("p (c w) -> p c w", c=c)

        wq_sb = _bv("wq", 4)
        wk_sb = _bv("wk", 4)
        wg_sb = _bv("wg", 4)
        wv_sb = _bv("wv", 3)
        wo_sb = _bv("wo", 4)
        anT = _bv("anT", 3)
        anTq = _bv("anTq", 3)
        bo_b = bob[:, 0:C]
        sb_mask = bob[:, C:C + KT]
        bgc = bob[:, C + KT:C + KT + 4]

        # mask bias per key partition (folded into the stored pair bias)
        mb = const.tile([128, KT], f32)
        nc.vector.tensor_scalar(mb, sb_mask, 1.0, INF, OP.subtract, OP.mult)
        ones_b = const.tile([128, CHP], b16)
        nc.vector.memset(ones_b, 1.0)

        # persistent SBUF state.  bias_sb is stored in the PSUM-native
        # (kt, qg, q', h) order, where q' = s*4+b is the transposed within-
        # group query index; the host permutes all q-indexed tensors to
        # match (PERM is an involution) and de-permutes the output rows.
        bias_sb = const.tile([128, KT, NQG, QG, H], b16)
        kTt = [const.tile([128, N], b16, name=f"kT{j}") for j in range(4)]
        v_aug = [const.tile([128, H, CHP], b16, name=f"vaug{t}") for t in range(KT)]
        qTt = [const.tile([128, NQ], b16, name=f"qT{j}") for j in range(4)]
        gT = const.tile([128, 4, NQ], b16)

        # ------------- pools (LIFO discipline: psp closes first, then
        # psA/psT after the z loop; attention PSUM pools open last) -------
        zpool = ctx.enter_context(tc.tile_pool(name="zpool", bufs=5))
        sbpool = ctx.enter_context(tc.tile_pool(name="sbp", bufs=4))
        scps = ctx.enter_context(tc.tile_pool(name="scps", bufs=3, space="PSUM"))
        pexp0 = ctx.enter_context(tc.tile_pool(name="pexp0", bufs=H))
        pexp = ctx.enter_context(tc.tile_pool(name="pexp", bufs=4))
        b_stack = ExitStack()
        psAp = b_stack.enter_context(tc.tile_pool(name="psA", bufs=2, space="PSUM"))
        psTp = b_stack.enter_context(tc.tile_pool(name="psT", bufs=2, space="PSUM"))
        a_stack = ExitStack()
        psp = a_stack.enter_context(tc.tile_pool(name="psproj", bufs=1, space="PSUM"))

        # ------------- phase B chunk emitters -------------
        _sbA = {}
        zgrp = {}

        def _zdma(g):
            # one DMA per DG chunks, alternating the two HWDGE queues
            # (gpsimd DMAs go through the slow software DGE path)
            zt_t = zpool.tile([CZ, DG, 2048], f8, tag="zt")
            nc.sync.dma_start(
                zt_t.rearrange("p a b -> p (a b)"),
                zt_d[:, 2048 * DG * g:2048 * DG * (g + 1)])
            zgrp[g] = zt_t

        _psA = {}

        def _chunk_mm(chk):
            zv = zgrp[chk // DG][:, chk % DG, :]
            ci = chk % 2
            if ci == 0:
                _psA[chk // 2] = psAp.tile([128, 512], f32, tag="psA",
                                           name="psA")
            psA = _psA[chk // 2]
            for b in range(NBLK):
                nc.tensor.matmul(
                    psA[32 * b:32 * b + 32, :], wzaug[:, ci, :],
                    zv[:, 512 * b:512 * (b + 1)],
                    start=(ci == 0), stop=(ci == 1),
                    tile_position=(0, 32 * b), skip_group_check=True,
                )
            if ci == 1:
                sbA = sbpool.tile([128, 512], b16, tag="sbA")
                nc.scalar.copy(sbA, psA)
                _sbA[chk // 2] = sbA

        def _pair_tail(pr):
            # chunks 2pr and 2pr+1 (same kt since NQG is even) share one
            # psA bank; 4 transposes cover both (rows 16ci..16ci+16 of each
            # quadrant hold chunk ci's heads)
            kt, qg0 = (2 * pr) // NQG, (2 * pr) % NQG
            sbA = _sbA.pop(pr)
            psT = psTp.tile([128, NBLK, NBLK, 32], b16, tag="psT")
            for s in range(NBLK):
                nc.tensor.transpose(
                    psT[:, s, :, :].rearrange("p a b -> p (a b)"),
                    sbA[:, 128 * s:128 * (s + 1)], sb_id,
                )
            # psT[:, s, b, 16ci+h] -> bias_sb[:, kt, qg0+ci, s*4+b, h]
            for ci in range(2):
                dst = bias_sb[:, kt, qg0 + ci, :, :].rearrange(
                    "p (s b) h -> p s b h", s=NBLK)
                nc.vector.tensor_scalar(
                    dst, psT[:, :, :, 16 * ci:16 * ci + H],
                    mb[:, kt:kt + 1], None, OP.add)

        # ------------- phase A emitters -------------
        def _piece_k(j, half):
            hw = 384
            kps = psp.tile([128, 384], f32, tag="pps", name=f"kps{j}_{half}")
            for c in range(3):
                nc.tensor.matmul(
                    kps,
                    wk_sb[:, c, 128 * j:128 * (j + 1)],
                    anT[:, c, hw * half:hw * (half + 1)],
                    start=(c == 0), stop=(c == 2),
                )
            nc.scalar.copy(kTt[j][:, hw * half:hw * (half + 1)], kps)

        def _piece_v(t):
            vps = psp.tile([128, C], f32, tag="pps", name="vps")
            for c in range(3):
                nc.tensor.matmul(
                    vps, anT[:, c, 128 * t:128 * (t + 1)], wv_sb[:, c, :],
                    start=(c == 0), stop=(c == 2),
                )
            nc.gpsimd.memset(v_aug[t], 0.0)
            nc.gpsimd.memset(v_aug[t][:, :, 0:1], 1.0)
            nc.vector.tensor_copy(
                v_aug[t][:, :, 1:CH + 1],
                vps.rearrange("p (h c) -> p h c", h=H),
            )

        def _piece_qg(j):
            # qk scale folded into Wq on the host
            qps = psp.tile([128, NQ], f32, tag="pps", name="qps")
            for c in range(3):
                nc.tensor.matmul(
                    qps, wq_sb[:, c, 128 * j:128 * (j + 1)], anTq[:, c, :],
                    start=(c == 0), stop=(c == 2),
                )
            nc.scalar.copy(qTt[j], qps)
            gps = psp.tile([128, NQ], f32, tag="pps", name="gps")
            for c in range(3):
                nc.tensor.matmul(
                    gps, wg_sb[:, c, 128 * j:128 * (j + 1)], anTq[:, c, :],
                    start=(c == 0), stop=(c == 2),
                )
            # sigmoid via the Exp table: g = 1/(1 + exp(-x - bg)), the -bg
            # arriving through the activation's per-partition bias port
            ge = sbpool.tile([128, NQ], f32, tag="ge", name=f"ge{j}")
            nc.scalar.activation(ge, gps, AF.Exp, scale=-1.0,
                                 bias=bgc[:, j:j + 1])
            gd = sbpool.tile([128, NQ], f32, tag="gd", name=f"gd{j}")
            nc.vector.tensor_scalar(gd, ge, 1.0, None, OP.add)
            with nc.allow_low_precision(reason="bf16 gating weights"):
                nc.vector.reciprocal(gT[:, j, :], gd)

        # ------------- phase C emitters -------------
        pt_t = {}

        def _scores(h, kg, add_on_pe=False):
            cn, j = h // 4, h % 4
            jb = 32 * j
            sc = scps.tile([128, KG, NQ], f32, tag="sc")
            bias = bias_sb[:, KG * kg:KG * (kg + 1), :, :, h]
            for ks in range(KG):
                kt = KG * kg + ks
                nc.tensor.matmul(
                    sc[:, ks, :],
                    kTt[cn][jb:jb + CHP, 128 * kt:128 * (kt + 1)],
                    qTt[cn][jb:jb + CHP, :],
                    start=(ks == 0), stop=(not add_on_pe and ks == KG - 1),
                    tile_position=(jb, 0), skip_group_check=True,
                )
            # pair+mask bias: identity matmul (PE) or strided add (DVE) —
            # the tail alternates so neither engine chains every head
            if add_on_pe:
                nc.tensor.matmul(
                    sc.rearrange("p a b -> p (a b)"), sb_id,
                    bias.rearrange("p t g q -> p (t g q)"),
                    start=False, stop=True, skip_group_check=True,
                )
            else:
                nc.vector.tensor_tensor(
                    sc, sc, bias.rearrange("p t g q -> p t (g q)"), OP.add)
            pool = pexp0 if kg == 0 else pexp
            p_t = pool.tile([128, KG, NQ], b16, tag="pt", name=f"pt{h}_{kg}")
            nc.scalar.activation(p_t, sc, AF.Exp)
            pt_t[(h, kg)] = p_t

        # ------------- emission schedule -------------
        pieces = (
            [lambda j=j: _piece_qg(j) for j in range(4)]
            + [lambda j=j, h=h: _piece_k(j, h) for h in range(2) for j in range(4)]
            + [lambda t=t: _piece_v(t) for t in range(KT)]
        )
        PC0 = 10                  # first phase-A piece slot
        SC0 = 20                  # first kg0 scores slot
        for g in range(4):        # prefetch 16 chunks of z
            _zdma(g)
        for chk in range(NCHUNK):
            if chk % DG == 0 and chk // DG + 4 < NCHUNK // DG:
                _zdma(chk // DG + 4)
            if chk < 3:
                _blob_dma(chk)
            elif chk % 2 == 1 and 3 + (chk - 3) // 2 < (BLOBW + BP - 1) // BP:
                _blob_dma(3 + (chk - 3) // 2)
            _chunk_mm(chk)
            if chk >= 3 and chk % 2 == 1:
                _pair_tail((chk - 3) // 2)
            s = chk - PC0
            if 0 <= s < len(pieces):
                pieces[s]()
            elif s == len(pieces):
                a_stack.close()
            if SC0 <= chk < SC0 + H:
                _scores(chk - SC0, 0)
        _pair_tail(NCHUNK // 2 - 1)
        b_stack.close()

        # ------------- post-loop: rest of kg0, then kg1 (lag-2) ------
        with (
            tc.tile_pool(name="otps", bufs=2, space="PSUM") as otps,
            tc.tile_pool(name="rbps", bufs=1, space="PSUM") as rbps,
            tc.tile_pool(name="psfin", bufs=1, space="PSUM") as psf,
            tc.tile_pool(name="tailp", bufs=2) as tailp,
        ):
            oT_t = {}
            goT = [const.tile([128, NQ], b16, name=f"goT{c}") for c in range(4)]
            ops = psf.tile([NQ, C], f32)

            def _pv6(h):
                cn, j = h // 4, h % 4
                jb = 32 * j
                if j == 0 and cn not in oT_t:
                    oT_t[cn] = otps.tile(
                        [128, NQ], f32, tag="oT", name=f"oT{cn}")
                for kg in range(2):
                    p_t = pt_t.pop((h, kg))
                    for ks in range(KG):
                        kt = KG * kg + ks
                        nc.tensor.matmul(
                            oT_t[cn][jb:jb + CHP, :], v_aug[kt][:, h, :],
                            p_t[:, ks, :],
                            start=(kt == 0), stop=(kt == KT - 1),
                            tile_position=(0, jb), skip_group_check=True,
                        )

            def _cn_tail(cn):
                oT = oT_t.pop(cn)
                rc = tailp.tile([128, NQ], b16, tag="rc", name=f"rc{cn}")
                with nc.allow_low_precision(reason="bf16 denominators"):
                    nc.vector.reciprocal(rc, oT)
                rb = rbps.tile([128, NQ], f32, tag="rb")
                for j in range(4):
                    jb = 32 * j
                    nc.tensor.matmul(
                        rb[jb:jb + CHP, :], ones_b[jb:jb + 1, :],
                        rc[jb:jb + 1, :],
                        tile_position=(jb, jb), skip_group_check=True,
                    )
                tmp = tailp.tile([128, NQ], f32, tag="tmp")
                nc.vector.tensor_tensor(tmp, oT, gT[:, cn, :], OP.mult)
                nc.vector.tensor_tensor(goT[cn], tmp, rb, OP.mult)
                nc.tensor.matmul(
                    ops, goT[cn], wo_sb[:, cn, :], start=(cn == 0),
                    stop=(cn == 3), skip_group_check=True,
                )

            for h in range(NCHUNK - SC0, H):
                _scores(h, 0)
            for h in range(H):
                if h >= 2:
                    _pv6(h - 2)
                _scores(h, 1)
                if h >= 2 and (h - 2) % 4 == 3:
                    _cn_tail((h - 2) // 4)
            _pv6(H - 2)
            _pv6(H - 1)
            _cn_tail(3)

            out_sb = tailp.tile([NQ, C], f32, tag="outsb")
            nc.vector.tensor_tensor(out_sb, ops, bo_b[0:NQ, :], OP.add)
            nc.sync.dma_start(out_d[:, :], out_sb)

    nc.compile()
    return nc


def _get_program():
    if "nc" not in _CACHE:
        _CACHE["nc"] = _build_program()
    return _CACHE["nc"]


def _pad_heads_cols(w, off):
    out = np.zeros((C, H, CHP), np.float32)
    out[:, :, off:off + CH] = np.asarray(w, np.float32).reshape(C, H, CH)
    return out.reshape(C, HP)


def _host_inputs(inputs):
    a = np.asarray(inputs["a"], np.float32)
    z = np.asarray(inputs["z"], np.float32)
    mask = np.asarray(inputs["mask"], np.float32)
    Wz = np.asarray(inputs["Wz"], np.float32)
    Wo = np.asarray(inputs["Wo"], np.float32)
    bg = np.asarray(inputs["bg"], np.float32)
    bo = np.asarray(inputs["bo"], np.float32)
    lnzw = np.asarray(inputs["ln_z_w"], np.float32)
    lnaw = np.asarray(inputs["ln_a_w"], np.float32)
    lnab = np.asarray(inputs["ln_a_b"], np.float32)

    # LN(a) folded on the host (elementwise only)
    mu = a.mean(axis=-1, keepdims=True)
    var = a.var(axis=-1, keepdims=True)
    an = ((a - mu) / np.sqrt(var + EPS) * lnaw + lnab)[0]   # [N, C]

    qscale = float(CH) ** -0.5
    Wq = qscale * np.asarray(inputs["Wq"], np.float32)
    Wk = np.asarray(inputs["Wk"], np.float32)
    Wg = np.asarray(inputs["Wg"], np.float32)
    Wv = np.asarray(inputs["Wv"], np.float32)

    wo_p = np.zeros((H, CHP, C), np.float32)
    wo_p[:, 1:CH + 1, :] = Wo.reshape(H, CH, C)
    bg_p = np.zeros((H, CHP), np.float32)
    bg_p[:, 1:CH + 1] = bg.reshape(H, CH)

    blob = np.zeros((128, BLOBW), np.float32)

    def _put3(nm, w):        # w: [384, width] -> [128, 3*width]
        o, tot = _BLOB[nm]
        width = tot // 3
        blob[:, o:o + tot] = w.reshape(3, 128, width).transpose(1, 0, 2).reshape(
            128, tot)

    def _put4(nm, w, width):  # w: [<=512, width] -> [128, 4*width]
        o, tot = _BLOB[nm]
        wp = np.zeros((4 * 128, width), np.float32)
        wp[:w.shape[0]] = w
        blob[:, o:o + tot] = wp.reshape(4, 128, width).transpose(1, 0, 2).reshape(
            128, tot)

    _put4("wq", _pad_heads_cols(Wq, 0), HP)
    _put4("wk", _pad_heads_cols(Wk, 0), HP)
    _put4("wg", _pad_heads_cols(Wg, 1), HP)
    _put3("wv", Wv)
    _put4("wo", wo_p.reshape(HP, C), C)
    _put3("anT", an.T.copy())            # anT[128c+p, t] -> [128, 3, 768]

    # centered fp8 half-stationaries [CZ, 2, 32]: [wza|0] and [0|wza]
    wzp = lnzw[:, None] * Wz
    wza_c = wzp - wzp.sum(axis=0, keepdims=True) / CZ
    wza = np.zeros((CZ, 2, 32), np.float32)
    wza[:, 0, 0:H] = wza_c
    wza[:, 1, H:2 * H] = wza_c

    bob = np.zeros((128, C + KT + 4), np.float32)
    bob[:, 0:C] = bo[None, :]
    bob[:, C:C + KT] = mask[0].reshape(KT, 128).T
    bob[:, C + KT:] = -bg_p.reshape(4, 128).T

    shared = {
        "blob_base": blob,
        "wza": wza.astype(float8_e4m3),
        "bob": bob,
        "ident": np.eye(128, dtype=bfloat16),
    }

    # z' = z * rstd, fp8, chunked kt-major for DoubleRow
    zf = z[0]
    zr = (zf * (1.0 / np.sqrt(zf.var(axis=-1) + EPS))[..., None]).astype(
        float8_e4m3)                      # [N(q), N(k), CZ]

    in_maps = []
    for core in range(NCORES):
        qs = slice(NQ * core, NQ * (core + 1))
        # [96, 768, 128] -> [CZ, KT, NQG, QG, 128k] -> [CZ, NCHUNK*2048]
        t = zr[qs].transpose(2, 1, 0).reshape(CZ, KT, 128, NQG, QG)
        t = t.transpose(0, 1, 3, 4, 2)
        zt = np.ascontiguousarray(t).reshape(CZ, NCHUNK * 2048)
        bl = blob.copy()
        o, tot = _BLOB["anTq"]
        bl[:, o:o + tot] = an[qs][PERM96].T.reshape(3, 128, NQ).transpose(
            1, 0, 2).reshape(128, tot)
        m = {k: v for k, v in shared.items() if k != "blob_base"}
        m["blob"] = bl.astype(bfloat16)
        m["zt"] = zt
        in_maps.append(m)
    return in_maps


def _run(inputs, trace=False):
    from concourse.bass_utils import run_bass_kernel_spmd

    nc = _get_program()
    in_maps = _host_inputs(inputs)
    res = run_bass_kernel_spmd(
        nc, in_maps, core_ids=list(range(NCORES)), trace=trace
    )
    rows = [res.results[i]["out"][PERM96] for i in range(NCORES)]
    out = np.concatenate(rows, axis=0).reshape(B, N, C).astype(np.float32)
    return out, res


def kernel(**inputs):
    out, _ = _run(inputs, trace=False)
    return out


# revision 104
# speedup vs baseline: 1.0408x; 1.0408x over previous
"""AttentionPairBias Trainium2 kernel (8 NeuronCores, SPMD over query rows).

Sharding: the 768 query rows are split 96-per-core. Each core computes the
full output rows for its query slice; the host concatenates.

Device-side math (per core). Both layer norms fold on the host
(elementwise / per-row reductions only, no contractions):
  an  = LN(a)                        shipped pre-transposed, bf16
  z'  = z * rstd(z)                  shipped fp8 (half the bytes of z+z^2)
  Wz'' = w*Wz - colsum(w*Wz)/CZ      so  pair_bias = z' @ Wz''  exactly
  (the LN-beta per-head constant cancels in the softmax and is dropped)

z pipeline: 36 chunks of [CZ, 16q x 128k].  Each chunk is 4 col-tiled fp8
matmuls (positions (0,32b) run concurrently on the PE); chunk PAIRS share
one psA bank via two half-stationaries [wza|0] / [0|wza] (even chunk
writes quadrant rows 0:16, odd accumulates rows 16:32), which halves both
the psA->sbuf copies and the PE transposes.  Transposed psT tiles land in
the PSUM-native (kt, qg, q'=s*4+b, h) order and are staged to SBUF by a
single contiguous vector op per pair; the host permutes every q-indexed
tensor by the same 4x4-transposition involution (PERM96) and de-permutes
output rows, so no strided writes exist anywhere in the bias path.

Attention: per (head, 3-kt group), qk matmuls accumulate into a scores
bank, the vector engine adds the staged bias (strided bf16 read), one Exp
per group, pv matmuls col-tiled per head quadrant.  kg0 scores+exp thread
into the back half of the z loop (p_t parks in SBUF); only the kg1 sweep,
pv, and per-cn tails (reciprocal via ones-broadcast matmul, gating from
the Exp-table sigmoid, Wo projection) run after the stream ends.

Scheduling notes:
 - all DMA queues matter: z' streams on the sync HWDGE queue as 9x 8KB-
   per-partition descriptors; the weight blob ships as 2048-col pieces on
   the scalar queue because the fabric round-robins queues per descriptor
   (oversized const descriptors starve the z stream, undersized blob
   pieces starve the blob)
 - z groups are prefetched 16 chunks deep; blob pieces trickle between
   them; phase-A projection pieces start at chunk 10 (qg first, then k
   halves kg0-first, then v) and all 16 kg0 score units thread in from
   chunk 20, so nothing blocks the in-order PE queue
 - gpsimd only memsets (it cannot touch PSUM and its DMAs go through the
   slow software DGE); the scalar table only ever holds Exp (one load)
"""

import os
import sys
import numpy as np

sys.path.insert(0, "/opt/trn_rl_repo")
os.environ.setdefault("MYCRO_LOCAL_CACHE", "1")

from ml_dtypes import bfloat16, float8_e4m3

# ---- problem constants (hardcoded per the harness contract) ----
B, N, C, CZ, H, CH = 1, 768, 384, 128, 16, 24
NCORES = 8
NQ = N // NCORES          # 96 query rows per core
CHP = 32                  # padded per-head width
HP = H * CHP              # 512 padded hc
EPS = 1e-5
INF = 1e9
KT = N // 128             # 6 key tiles
QG = 16                   # query rows per z-chunk
NQG = NQ // QG            # 6 query groups
NCHUNK = KT * NQG         # 36 chunks, key-tile major
NBLK = 4                  # 512-wide output blocks per chunk
DG = 4                    # chunks per z DMA (8KB/partition descriptors)
KG = 3                    # key tiles per scores group
# within-16 query transposition (involution): device col p <-> query 4(p%4)+p//4
PERM96 = np.arange(NQ).reshape(NQG, 4, 4).transpose(0, 2, 1).reshape(NQ)

# bf16 constant blob layout (columns, all [128, x]).  Order matters: the
# first section ships in an early DMA (k/q/g projections unblock first).
_BLOB = {}
_off = 0
for _nm, _w in [("anTq", 3 * NQ), ("wq", 4 * HP), ("wg", 4 * HP),
                ("anT", 3 * N), ("wk", 4 * HP),
                ("wv", 3 * C), ("wo", 4 * C)]:
    _BLOB[_nm] = (_off, _w)
    _off += _w
BLOBW = _off

_CACHE = {}


def _build_program():
    from contextlib import ExitStack
    import concourse.bass as bass
    import concourse.tile as tile
    from concourse import bacc, mybir

    f32 = mybir.dt.float32
    b16 = mybir.dt.bfloat16
    f8 = mybir.dt.float8e4
    AF = mybir.ActivationFunctionType
    OP = mybir.AluOpType

    nc = bacc.Bacc("TRN2", target_bir_lowering=False, debug=False)

    # ---- DRAM I/O ----
    # z' fp8 chunks: per chunk [CZ, 2048] (16 q x 128 k, k minor)
    zt_d = nc.dram_tensor("zt", [CZ, NCHUNK * 2048], f8, kind="ExternalInput")
    # two half-stationaries: [wza|0] and [0|wza] — chunk pairs share one
    # psA bank (even chunk writes quadrant rows 0:16, odd accumulates 16:32)
    wza_d = nc.dram_tensor("wza", [CZ, 2, 32], f8, kind="ExternalInput")
    id_d = nc.dram_tensor("ident", [128, 128], b16, kind="ExternalInput")
    blob_d = nc.dram_tensor("blob", [128, BLOBW], b16, kind="ExternalInput")
    # f32 sidecar: bo row | mask cols | -bg activation-bias cols
    bob_d = nc.dram_tensor("bob", [128, C + KT + 4], f32, kind="ExternalInput")
    out_d = nc.dram_tensor("out", [NQ, C], f32, kind="ExternalOutput")

    with tile.TileContext(nc) as tc, ExitStack() as ctx:
        const = ctx.enter_context(tc.tile_pool(name="const", bufs=1))

        # ------------- constant loads.  The blob ships as 2048-col pieces
        # so its DMA descriptors match the z stream's (the fabric round-
        # robins by descriptor; oversized const descriptors starve z) ------
        wzaug = const.tile([CZ, 2, 32], f8)
        nc.scalar.dma_start(wzaug, wza_d[:, :, :])
        sb_id = const.tile([128, 128], b16)
        nc.scalar.dma_start(sb_id, id_d[:, :])
        bob = const.tile([128, C + KT + 4], f32)
        nc.scalar.dma_start(bob, bob_d[:, :])
        blob = const.tile([128, BLOBW], b16)
        BP = 2048

        def _blob_dma(i):
            lo, hi = BP * i, min(BP * (i + 1), BLOBW)
            nc.scalar.dma_start(blob[:, lo:hi], blob_d[:, lo:hi])

        for _bi in range(3):
            _blob_dma(_bi)

        def _bv(nm, c):
            o, w = _BLOB[nm]
            return blob[:, o:o + w].rearrange("p (c w) -> p c w", c=c)

        wq_sb = _bv("wq", 4)
        wk_sb = _bv("wk", 4)
        wg_sb = _bv("wg", 4)
        wv_sb = _bv("wv", 3)
        wo_sb = _bv("wo", 4)
        anT = _bv("anT", 3)
        anTq = _bv("anTq", 3)
        bo_b = bob[:, 0:C]
        sb_mask = bob[:, C:C + KT]
        bgc = bob[:, C + KT:C + KT + 4]

        # mask bias per key partition (folded into the stored pair bias)
        mb = const.tile([128, KT], f32)
        nc.vector.tensor_scalar(mb, sb_mask, 1.0, INF, OP.subtract, OP.mult)
        ones_b = const.tile([128, CHP], b16)
        nc.vector.memset(ones_b, 1.0)

        # persistent SBUF state.  bias_sb is stored in the PSUM-native
        # (kt, qg, q', h) order, where q' = s*4+b is the transposed within-
        # group query index; the host permutes all q-indexed tensors to
        # match (PERM is an involution) and de-permutes the output rows.
        bias_sb = const.tile([128, KT, NQG, QG, H], b16)
        kTt = [const.tile([128, N], b16, name=f"kT{j}") for j in range(4)]
        v_aug = [const.tile([128, H, CHP], b16, name=f"vaug{t}") for t in range(KT)]
        qTt = [const.tile([128, NQ], b16, name=f"qT{j}") for j in range(4)]
        gT = const.tile([128, 4, NQ], b16)

        # ------------- pools (LIFO discipline: psp closes first, then
        # psA/psT after the z loop; attention PSUM pools open last) -------
        zpool = ctx.enter_context(tc.tile_pool(name="zpool", bufs=5))
        sbpool = ctx.enter_context(tc.tile_pool(name="sbp", bufs=4))
        scps = ctx.enter_context(tc.tile_pool(name="scps", bufs=3, space="PSUM"))
        pexp0 = ctx.enter_context(tc.tile_pool(name="pexp0", bufs=H))
        pexp = ctx.enter_context(tc.tile_pool(name="pexp", bufs=4))
        b_stack = ExitStack()
        psAp = b_stack.enter_context(tc.tile_pool(name="psA", bufs=2, space="PSUM"))
        psTp = b_stack.enter_context(tc.tile_pool(name="psT", bufs=2, space="PSUM"))
        a_stack = ExitStack()
        psp = a_stack.enter_context(tc.tile_pool(name="psproj", bufs=1, space="PSUM"))

        # ------------- phase B chunk emitters -------------
        _sbA = {}
        zgrp = {}

        def _zdma(g):
            # one DMA per DG chunks, alternating the two HWDGE queues
            # (gpsimd DMAs go through the slow software DGE path)
            zt_t = zpool.tile([CZ, DG, 2048], f8, tag="zt")
            nc.sync.dma_start(
                zt_t.rearrange("p a b -> p (a b)"),
                zt_d[:, 2048 * DG * g:2048 * DG * (g + 1)])
            zgrp[g] = zt_t

        _psA = {}

        def _chunk_mm(chk):
            zv = zgrp[chk // DG][:, chk % DG, :]
            ci = chk % 2
            if ci == 0:
                _psA[chk // 2] = psAp.tile([128, 512], f32, tag="psA",
                                           name="psA")
            psA = _psA[chk // 2]
            for b in range(NBLK):
                nc.tensor.matmul(
                    psA[32 * b:32 * b + 32, :], wzaug[:, ci, :],
                    zv[:, 512 * b:512 * (b + 1)],
                    start=(ci == 0), stop=(ci == 1),
                    tile_position=(0, 32 * b), skip_group_check=True,
                )
            if ci == 1:
                sbA = sbpool.tile([128, 512], b16, tag="sbA")
                nc.scalar.copy(sbA, psA)
                _sbA[chk // 2] = sbA

        def _pair_tail(pr):
            # chunks 2pr and 2pr+1 (same kt since NQG is even) share one
            # psA bank; 4 transposes cover both (rows 16ci..16ci+16 of each
            # quadrant hold chunk ci's heads)
            kt, qg0 = (2 * pr) // NQG, (2 * pr) % NQG
            sbA = _sbA.pop(pr)
            psT = psTp.tile([128, NBLK, NBLK, 32], b16, tag="psT")
            for s in range(NBLK):
                nc.tensor.transpose(
                    psT[:, s, :, :].rearrange("p a b -> p (a b)"),
                    sbA[:, 128 * s:128 * (s + 1)], sb_id,
                )
            # psT[:, s, b, 16ci+h] -> bias_sb[:, kt, qg0+ci, s*4+b, h]
            for ci in range(2):
                dst = bias_sb[:, kt, qg0 + ci, :, :].rearrange(
                    "p (s b) h -> p s b h", s=NBLK)
                nc.vector.tensor_scalar(
                    dst, psT[:, :, :, 16 * ci:16 * ci + H],
                    mb[:, kt:kt + 1], None, OP.add)

        # ------------- phase A emitters -------------
        def _piece_k(j, half):
            hw = 384
            kps = psp.tile([128, 384], f32, tag="pps", name=f"kps{j}_{half}")
            for c in range(3):
                nc.tensor.matmul(
                    kps,
                    wk_sb[:, c, 128 * j:128 * (j + 1)],
                    anT[:, c, hw * half:hw * (half + 1)],
                    start=(c == 0), stop=(c == 2),
                )
            nc.scalar.copy(kTt[j][:, hw * half:hw * (half + 1)], kps)

        def _piece_v(t):
            vps = psp.tile([128, C], f32, tag="pps", name="vps")
            for c in range(3):
                nc.tensor.matmul(
                    vps, anT[:, c, 128 * t:128 * (t + 1)], wv_sb[:, c, :],
                    start=(c == 0), stop=(c == 2),
                )
            nc.gpsimd.memset(v_aug[t], 0.0)
            nc.gpsimd.memset(v_aug[t][:, :, 0:1], 1.0)
            nc.vector.tensor_copy(
                v_aug[t][:, :, 1:CH + 1],
                vps.rearrange("p (h c) -> p h c", h=H),
            )

        def _piece_qg(j):
            # qk scale folded into Wq on the host
            qps = psp.tile([128, NQ], f32, tag="pps", name="qps")
            for c in range(3):
                nc.tensor.matmul(
                    qps, wq_sb[:, c, 128 * j:128 * (j + 1)], anTq[:, c, :],
                    start=(c == 0), stop=(c == 2),
                )
            nc.scalar.copy(qTt[j], qps)
            gps = psp.tile([128, NQ], f32, tag="pps", name="gps")
            for c in range(3):
                nc.tensor.matmul(
                    gps, wg_sb[:, c, 128 * j:128 * (j + 1)], anTq[:, c, :],
                    start=(c == 0), stop=(c == 2),
                )
            # sigmoid via the Exp table: g = 1/(1 + exp(-x - bg)), the -bg
            # arriving through the activation's per-partition bias port
            ge = sbpool.tile([128, NQ], f32, tag="ge", name=f"ge{j}")
            nc.scalar.activation(ge, gps, AF.Exp, scale=-1.0,
                                 bias=bgc[:, j:j + 1])
            gd = sbpool.tile([128, NQ], f32, tag="gd", name=f"gd{j}")
            nc.vector.tensor_scalar(gd, ge, 1.0, None, OP.add)
            with nc.allow_low_precision(reason="bf16 gating weights"):
                nc.vector.reciprocal(gT[:, j, :], gd)

        # ------------- phase C emitters -------------
        pt_t = {}

        def _scores(h, kg, add_on_pe=False):
            cn, j = h // 4, h % 4
            jb = 32 * j
            sc = scps.tile([128, KG, NQ], f32, tag="sc")
            bias = bias_sb[:, KG * kg:KG * (kg + 1), :, :, h]
            for ks in range(KG):
                kt = KG * kg + ks
                nc.tensor.matmul(
                    sc[:, ks, :],
                    kTt[cn][jb:jb + CHP, 128 * kt:128 * (kt + 1)],
                    qTt[cn][jb:jb + CHP, :],
                    start=(ks == 0), stop=(not add_on_pe and ks == KG - 1),
                    tile_position=(jb, 0), skip_group_check=True,
                )
            # pair+mask bias: identity matmul (PE) or strided add (DVE) —
            # the tail alternates so neither engine chains every head
            if add_on_pe:
                nc.tensor.matmul(
                    sc.rearrange("p a b -> p (a b)"), sb_id,
                    bias.rearrange("p t g q -> p (t g q)"),
                    start=False, stop=True, skip_group_check=True,
                )
            else:
                nc.vector.tensor_tensor(
                    sc, sc, bias.rearrange("p t g q -> p t (g q)"), OP.add)
            pool = pexp0 if kg == 0 else pexp
            p_t = pool.tile([128, KG, NQ], b16, tag="pt", name=f"pt{h}_{kg}")
            nc.scalar.activation(p_t, sc, AF.Exp)
            pt_t[(h, kg)] = p_t

        # ------------- emission schedule -------------
        pieces = (
            [lambda j=j: _piece_qg(j) for j in range(4)]
            + [lambda j=j, h=h: _piece_k(j, h) for h in range(2) for j in range(4)]
            + [lambda t=t: _piece_v(t) for t in range(KT)]
        )
        PC0 = 10                  # first phase-A piece slot
        SC0 = 20                  # first kg0 scores slot
        for g in range(4):        # prefetch 16 chunks of z
            _zdma(g)
        for chk in range(NCHUNK):
            if chk % DG == 0 and chk // DG + 4 < NCHUNK // DG:
                _zdma(chk // DG + 4)
            if chk % 2 == 1 and 3 + chk // 2 < (BLOBW + BP - 1) // BP:
                _blob_dma(3 + chk // 2)
            _chunk_mm(chk)
            if chk >= 3 and chk % 2 == 1:
                _pair_tail((chk - 3) // 2)
            s = chk - PC0
            if 0 <= s < len(pieces):
                pieces[s]()
            elif s == len(pieces):
                a_stack.close()
            if SC0 <= chk < SC0 + H:
                _scores(chk - SC0, 0)
        _pair_tail(NCHUNK // 2 - 1)
        b_stack.close()

        # ------------- post-loop: rest of kg0, then kg1 (lag-2) ------
        with (
            tc.tile_pool(name="otps", bufs=2, space="PSUM") as otps,
            tc.tile_pool(name="rbps", bufs=1, space="PSUM") as rbps,
            tc.tile_pool(name="psfin", bufs=1, space="PSUM") as psf,
            tc.tile_pool(name="tailp", bufs=2) as tailp,
        ):
            oT_t = {}
            goT = [const.tile([128, NQ], b16, name=f"goT{c}") for c in range(4)]
            ops = psf.tile([NQ, C], f32)

            def _pv6(h):
                cn, j = h // 4, h % 4
                jb = 32 * j
                if j == 0 and cn not in oT_t:
                    oT_t[cn] = otps.tile(
                        [128, NQ], f32, tag="oT", name=f"oT{cn}")
                for kg in range(2):
                    p_t = pt_t.pop((h, kg))
                    for ks in range(KG):
                        kt = KG * kg + ks
                        nc.tensor.matmul(
                            oT_t[cn][jb:jb + CHP, :], v_aug[kt][:, h, :],
                            p_t[:, ks, :],
                            start=(kt == 0), stop=(kt == KT - 1),
                            tile_position=(0, jb), skip_group_check=True,
                        )

            def _cn_tail(cn):
                oT = oT_t.pop(cn)
                rc = tailp.tile([128, NQ], b16, tag="rc", name=f"rc{cn}")
                with nc.allow_low_precision(reason="bf16 denominators"):
                    nc.vector.reciprocal(rc, oT)
                rb = rbps.tile([128, NQ], f32, tag="rb")
                for j in range(4):
                    jb = 32 * j
                    nc.tensor.matmul(
                        rb[jb:jb + CHP, :], ones_b[jb:jb + 1, :],
                        rc[jb:jb + 1, :],
                        tile_position=(jb, jb), skip_group_check=True,
                    )
                tmp = tailp.tile([128, NQ], f32, tag="tmp")
                nc.vector.tensor_tensor(tmp, oT, gT[:, cn, :], OP.mult)
                nc.vector.tensor_tensor(goT[cn], tmp, rb, OP.mult)
                nc.tensor.matmul(
                    ops, goT[cn], wo_sb[:, cn, :], start=(cn == 0),
                    stop=(cn == 3), skip_group_check=True,
                )

            for h in range(NCHUNK - SC0, H):
                _scores(h, 0)
            for h in range(H):
                if h >= 2:
                    _pv6(h - 2)
                _scores(h, 1)
                if h >= 2 and (h - 2) % 4 == 3:
                    _cn_tail((h - 2) // 4)
            _pv6(H - 2)
            _pv6(H - 1)
            _cn_tail(3)

            out_sb = tailp.tile([NQ, C], f32, tag="outsb")
            nc.vector.tensor_tensor(out_sb, ops, bo_b[0:NQ, :], OP.add)
            nc.sync.dma_start(out_d[:, :], out_sb)

    nc.compile()
    return nc


def _get_program():
    if "nc" not in _CACHE:
        _CACHE["nc"] = _build_program()
    return _CACHE["nc"]


def _pad_heads_cols(w, off):
    out = np.zeros((C, H, CHP), np.float32)
    out[:, :, off:off + CH] = np.asarray(w, np.float32).reshape(C, H, CH)
    return out.reshape(C, HP)


def _host_inputs(inputs):
    a = np.asarray(inputs["a"], np.float32)
    z = np.asarray(inputs["z"], np.float32)
    mask = np.asarray(inputs["mask"], np.float32)
    Wz = np.asarray(inputs["Wz"], np.float32)
    Wo = np.asarray(inputs["Wo"], np.float32)
    bg = np.asarray(inputs["bg"], np.float32)
    bo = np.asarray(inputs["bo"], np.float32)
    lnzw = np.asarray(inputs["ln_z_w"], np.float32)
    lnaw = np.asarray(inputs["ln_a_w"], np.float32)
    lnab = np.asarray(inputs["ln_a_b"], np.float32)

    # LN(a) folded on the host (elementwise only)
    mu = a.mean(axis=-1, keepdims=True)
    var = a.var(axis=-1, keepdims=True)
    an = ((a - mu) / np.sqrt(var + EPS) * lnaw + lnab)[0]   # [N, C]

    qscale = float(CH) ** -0.5
    Wq = qscale * np.asarray(inputs["Wq"], np.float32)
    Wk = np.asarray(inputs["Wk"], np.float32)
    Wg = np.asarray(inputs["Wg"], np.float32)
    Wv = np.asarray(inputs["Wv"], np.float32)

    wo_p = np.zeros((H, CHP, C), np.float32)
    wo_p[:, 1:CH + 1, :] = Wo.reshape(H, CH, C)
    bg_p = np.zeros((H, CHP), np.float32)
    bg_p[:, 1:CH + 1] = bg.reshape(H, CH)

    blob = np.zeros((128, BLOBW), np.float32)

    def _put3(nm, w):        # w: [384, width] -> [128, 3*width]
        o, tot = _BLOB[nm]
        width = tot // 3
        blob[:, o:o + tot] = w.reshape(3, 128, width).transpose(1, 0, 2).reshape(
            128, tot)

    def _put4(nm, w, width):  # w: [<=512, width] -> [128, 4*width]
        o, tot = _BLOB[nm]
        wp = np.zeros((4 * 128, width), np.float32)
        wp[:w.shape[0]] = w
        blob[:, o:o + tot] = wp.reshape(4, 128, width).transpose(1, 0, 2).reshape(
            128, tot)

    _put4("wq", _pad_heads_cols(Wq, 0), HP)
    _put4("wk", _pad_heads_cols(Wk, 0), HP)
    _put4("wg", _pad_heads_cols(Wg, 1), HP)
    _put3("wv", Wv)
    _put4("wo", wo_p.reshape(HP, C), C)
    _put3("anT", an.T.copy())            # anT[128c+p, t] -> [128, 3, 768]

    # centered fp8 half-stationaries [CZ, 2, 32]: [wza|0] and [0|wza]
    wzp = lnzw[:, None] * Wz
    wza_c = wzp - wzp.sum(axis=0, keepdims=True) / CZ
    wza = np.zeros((CZ, 2, 32), np.float32)
    wza[:, 0, 0:H] = wza_c
    wza[:, 1, H:2 * H] = wza_c

    bob = np.zeros((128, C + KT + 4), np.float32)
    bob[:, 0:C] = bo[None, :]
    bob[:, C:C + KT] = mask[0].reshape(KT, 128).T
    bob[:, C + KT:] = -bg_p.reshape(4, 128).T

    shared = {
        "blob_base": blob,
        "wza": wza.astype(float8_e4m3),
        "bob": bob,
        "ident": np.eye(128, dtype=bfloat16),
    }

    # z' = z * rstd, fp8, chunked kt-major for DoubleRow
    zf = z[0]
    zr = (zf * (1.0 / np.sqrt(zf.var(axis=-1) + EPS))[..., None]).astype(
        float8_e4m3)                      # [N(q), N(k), CZ]

    in_maps = []
    for core in range(NCORES):
        qs = slice(NQ * core, NQ * (core + 1))
        # [96, 768, 128] -> [CZ, KT, NQG, QG, 128k] -> [CZ, NCHUNK*2048]
        t = zr[qs].transpose(2, 1, 0).reshape(CZ, KT, 128, NQG, QG)
        t = t.transpose(0, 1, 3, 4, 2)
        zt = np.ascontiguousarray(t).reshape(CZ, NCHUNK * 2048)
        bl = blob.copy()
        o, tot = _BLOB["anTq"]
        bl[:, o:o + tot] = an[qs][PERM96].T.reshape(3, 128, NQ).transpose(
            1, 0, 2).reshape(128, tot)
        m = {k: v for k, v in shared.items() if k != "blob_base"}
        m["blob"] = bl.astype(bfloat16)
        m["zt"] = zt
        in_maps.append(m)
    return in_maps


def _run(inputs, trace=False):
    from concourse.bass_utils import run_bass_kernel_spmd

    nc = _get_program()
    in_maps = _host_inputs(inputs)
    res = run_bass_kernel_spmd(
        nc, in_maps, core_ids=list(range(NCORES)), trace=trace
    )
    rows = [res.results[i]["out"][PERM96] for i in range(NCORES)]
    out = np.concatenate(rows, axis=0).reshape(B, N, C).astype(np.float32)
    return out, res


def kernel(**inputs):
    out, _ = _run(inputs, trace=False)
    return out


# revision 105
# speedup vs baseline: 1.0442x; 1.0032x over previous
"""AttentionPairBias Trainium2 kernel (8 NeuronCores, SPMD over query rows).

Sharding: the 768 query rows are split 96-per-core. Each core computes the
full output rows for its query slice; the host concatenates.

Device-side math (per core). Both layer norms fold on the host
(elementwise / per-row reductions only, no contractions):
  an  = LN(a)                        shipped pre-transposed, bf16
  z'  = z * rstd(z)                  shipped fp8 (half the bytes of z+z^2)
  Wz'' = w*Wz - colsum(w*Wz)/CZ      so  pair_bias = z' @ Wz''  exactly
  (the LN-beta per-head constant cancels in the softmax and is dropped)

z pipeline: 36 chunks of [CZ, 16q x 128k].  Each chunk is 4 col-tiled fp8
matmuls (positions (0,32b) run concurrently on the PE); chunk PAIRS share
one psA bank via two half-stationaries [wza|0] / [0|wza] (even chunk
writes quadrant rows 0:16, odd accumulates rows 16:32), which halves both
the psA->sbuf copies and the PE transposes.  Transposed psT tiles land in
the PSUM-native (kt, qg, q'=s*4+b, h) order and are staged to SBUF by a
single contiguous vector op per pair; the host permutes every q-indexed
tensor by the same 4x4-transposition involution (PERM96) and de-permutes
output rows, so no strided writes exist anywhere in the bias path.

Attention: per (head, 3-kt group), qk matmuls accumulate into a scores
bank, the vector engine adds the staged bias (strided bf16 read), one Exp
per group, pv matmuls col-tiled per head quadrant.  kg0 scores+exp thread
into the back half of the z loop (p_t parks in SBUF); only the kg1 sweep,
pv, and per-cn tails (reciprocal via ones-broadcast matmul, gating from
the Exp-table sigmoid, Wo projection) run after the stream ends.

Scheduling notes:
 - all DMA queues matter: z' streams on the sync HWDGE queue as 9x 8KB-
   per-partition descriptors; the weight blob ships as 2048-col pieces on
   the scalar queue because the fabric round-robins queues per descriptor
   (oversized const descriptors starve the z stream, undersized blob
   pieces starve the blob)
 - z groups are prefetched 16 chunks deep; blob pieces trickle between
   them; phase-A projection pieces start at chunk 10 (qg first, then k
   halves kg0-first, then v) and all 16 kg0 score units thread in from
   chunk 20, so nothing blocks the in-order PE queue
 - gpsimd only memsets (it cannot touch PSUM and its DMAs go through the
   slow software DGE); the scalar table only ever holds Exp (one load)
"""

import os
import sys
import numpy as np

sys.path.insert(0, "/opt/trn_rl_repo")
os.environ.setdefault("MYCRO_LOCAL_CACHE", "1")

from ml_dtypes import bfloat16, float8_e4m3

# ---- problem constants (hardcoded per the harness contract) ----
B, N, C, CZ, H, CH = 1, 768, 384, 128, 16, 24
NCORES = 8
NQ = N // NCORES          # 96 query rows per core
CHP = 32                  # padded per-head width
HP = H * CHP              # 512 padded hc
EPS = 1e-5
INF = 1e9
KT = N // 128             # 6 key tiles
QG = 16                   # query rows per z-chunk
NQG = NQ // QG            # 6 query groups
NCHUNK = KT * NQG         # 36 chunks, key-tile major
NBLK = 4                  # 512-wide output blocks per chunk
DG = 4                    # chunks per z DMA (8KB/partition descriptors)
KG = 3                    # key tiles per scores group
# within-16 query transposition (involution): device col p <-> query 4(p%4)+p//4
PERM96 = np.arange(NQ).reshape(NQG, 4, 4).transpose(0, 2, 1).reshape(NQ)

# bf16 constant blob layout (columns, all [128, x]).  Order matters: the
# first section ships in an early DMA (k/q/g projections unblock first).
_BLOB = {}
_off = 0
for _nm, _w in [("anTq", 3 * NQ), ("wq", 4 * HP), ("wg", 4 * HP),
                ("anT", 3 * N), ("wk", 4 * HP),
                ("wv", 3 * C), ("wo", 4 * C)]:
    _BLOB[_nm] = (_off, _w)
    _off += _w
BLOBW = _off

_CACHE = {}


def _build_program():
    from contextlib import ExitStack
    import concourse.bass as bass
    import concourse.tile as tile
    from concourse import bacc, mybir

    f32 = mybir.dt.float32
    b16 = mybir.dt.bfloat16
    f8 = mybir.dt.float8e4
    AF = mybir.ActivationFunctionType
    OP = mybir.AluOpType

    nc = bacc.Bacc("TRN2", target_bir_lowering=False, debug=False)

    # ---- DRAM I/O ----
    # z' fp8 chunks: per chunk [CZ, 2048] (16 q x 128 k, k minor)
    zt_d = nc.dram_tensor("zt", [CZ, NCHUNK * 2048], f8, kind="ExternalInput")
    # two half-stationaries: [wza|0] and [0|wza] — chunk pairs share one
    # psA bank (even chunk writes quadrant rows 0:16, odd accumulates 16:32)
    wza_d = nc.dram_tensor("wza", [CZ, 2, 32], f8, kind="ExternalInput")
    id_d = nc.dram_tensor("ident", [128, 128], b16, kind="ExternalInput")
    blob_d = nc.dram_tensor("blob", [128, BLOBW], b16, kind="ExternalInput")
    # f32 sidecar: bo row | mask cols | -bg activation-bias cols
    bob_d = nc.dram_tensor("bob", [128, C + KT + 4], f32, kind="ExternalInput")
    out_d = nc.dram_tensor("out", [NQ, C], f32, kind="ExternalOutput")

    with tile.TileContext(nc) as tc, ExitStack() as ctx:
        const = ctx.enter_context(tc.tile_pool(name="const", bufs=1))

        # ------------- constant loads.  The blob ships as 2048-col pieces
        # so its DMA descriptors match the z stream's (the fabric round-
        # robins by descriptor; oversized const descriptors starve z) ------
        wzaug = const.tile([CZ, 2, 32], f8)
        nc.scalar.dma_start(wzaug, wza_d[:, :, :])
        sb_id = const.tile([128, 128], b16)
        nc.scalar.dma_start(sb_id, id_d[:, :])
        bob = const.tile([128, C + KT + 4], f32)
        nc.scalar.dma_start(bob, bob_d[:, :])
        blob = const.tile([128, BLOBW], b16)
        BP = 2048

        def _blob_dma(i):
            lo, hi = BP * i, min(BP * (i + 1), BLOBW)
            nc.scalar.dma_start(blob[:, lo:hi], blob_d[:, lo:hi])

        for _bi in range(3):
            _blob_dma(_bi)

        def _bv(nm, c):
            o, w = _BLOB[nm]
            return blob[:, o:o + w].rearrange("p (c w) -> p c w", c=c)

        wq_sb = _bv("wq", 4)
        wk_sb = _bv("wk", 4)
        wg_sb = _bv("wg", 4)
        wv_sb = _bv("wv", 3)
        wo_sb = _bv("wo", 4)
        anT = _bv("anT", 3)
        anTq = _bv("anTq", 3)
        bo_b = bob[:, 0:C]
        sb_mask = bob[:, C:C + KT]
        bgc = bob[:, C + KT:C + KT + 4]

        # mask bias per key partition (folded into the stored pair bias)
        mb = const.tile([128, KT], f32)
        nc.vector.tensor_scalar(mb, sb_mask, 1.0, INF, OP.subtract, OP.mult)
        ones_b = const.tile([128, CHP], b16)
        nc.vector.memset(ones_b, 1.0)

        # persistent SBUF state.  bias_sb is stored in the PSUM-native
        # (kt, qg, q', h) order, where q' = s*4+b is the transposed within-
        # group query index; the host permutes all q-indexed tensors to
        # match (PERM is an involution) and de-permutes the output rows.
        bias_sb = const.tile([128, KT, NQG, QG, H], b16)
        kTt = [const.tile([128, N], b16, name=f"kT{j}") for j in range(4)]
        v_aug = [const.tile([128, H, CHP], b16, name=f"vaug{t}") for t in range(KT)]
        qTt = [const.tile([128, NQ], b16, name=f"qT{j}") for j in range(4)]
        gT = const.tile([128, 4, NQ], b16)

        # ------------- pools (LIFO discipline: psp closes first, then
        # psA/psT after the z loop; attention PSUM pools open last) -------
        zpool = ctx.enter_context(tc.tile_pool(name="zpool", bufs=5))
        sbpool = ctx.enter_context(tc.tile_pool(name="sbp", bufs=4))
        scps = ctx.enter_context(tc.tile_pool(name="scps", bufs=4, space="PSUM"))
        pexp0 = ctx.enter_context(tc.tile_pool(name="pexp0", bufs=H))
        pexp = ctx.enter_context(tc.tile_pool(name="pexp", bufs=4))
        b_stack = ExitStack()
        psAp = b_stack.enter_context(tc.tile_pool(name="psA", bufs=2, space="PSUM"))
        psTp = b_stack.enter_context(tc.tile_pool(name="psT", bufs=1, space="PSUM"))
        a_stack = ExitStack()
        psp = a_stack.enter_context(tc.tile_pool(name="psproj", bufs=1, space="PSUM"))

        # ------------- phase B chunk emitters -------------
        _sbA = {}
        zgrp = {}

        def _zdma(g):
            # one DMA per DG chunks, alternating the two HWDGE queues
            # (gpsimd DMAs go through the slow software DGE path)
            zt_t = zpool.tile([CZ, DG, 2048], f8, tag="zt")
            nc.sync.dma_start(
                zt_t.rearrange("p a b -> p (a b)"),
                zt_d[:, 2048 * DG * g:2048 * DG * (g + 1)])
            zgrp[g] = zt_t

        _psA = {}

        def _chunk_mm(chk):
            zv = zgrp[chk // DG][:, chk % DG, :]
            ci = chk % 2
            if ci == 0:
                _psA[chk // 2] = psAp.tile([128, 512], f32, tag="psA",
                                           name="psA")
            psA = _psA[chk // 2]
            for b in range(NBLK):
                nc.tensor.matmul(
                    psA[32 * b:32 * b + 32, :], wzaug[:, ci, :],
                    zv[:, 512 * b:512 * (b + 1)],
                    start=(ci == 0), stop=(ci == 1),
                    tile_position=(0, 32 * b), skip_group_check=True,
                )
            if ci == 1:
                sbA = sbpool.tile([128, 512], b16, tag="sbA")
                nc.scalar.copy(sbA, psA)
                _sbA[chk // 2] = sbA

        def _pair_tail(pr):
            # chunks 2pr and 2pr+1 (same kt since NQG is even) share one
            # psA bank; 4 transposes cover both (rows 16ci..16ci+16 of each
            # quadrant hold chunk ci's heads)
            kt, qg0 = (2 * pr) // NQG, (2 * pr) % NQG
            sbA = _sbA.pop(pr)
            psT = psTp.tile([128, NBLK, NBLK, 32], b16, tag="psT")
            for s in range(NBLK):
                nc.tensor.transpose(
                    psT[:, s, :, :].rearrange("p a b -> p (a b)"),
                    sbA[:, 128 * s:128 * (s + 1)], sb_id,
                )
            # psT[:, s, b, 16ci+h] -> bias_sb[:, kt, qg0+ci, s*4+b, h]
            for ci in range(2):
                dst = bias_sb[:, kt, qg0 + ci, :, :].rearrange(
                    "p (s b) h -> p s b h", s=NBLK)
                nc.vector.tensor_scalar(
                    dst, psT[:, :, :, 16 * ci:16 * ci + H],
                    mb[:, kt:kt + 1], None, OP.add)

        # ------------- phase A emitters -------------
        def _piece_k(j, half):
            hw = 384
            kps = psp.tile([128, 384], f32, tag="pps", name=f"kps{j}_{half}")
            for c in range(3):
                nc.tensor.matmul(
                    kps,
                    wk_sb[:, c, 128 * j:128 * (j + 1)],
                    anT[:, c, hw * half:hw * (half + 1)],
                    start=(c == 0), stop=(c == 2),
                )
            nc.scalar.copy(kTt[j][:, hw * half:hw * (half + 1)], kps)

        def _piece_v(t):
            vps = psp.tile([128, C], f32, tag="pps", name="vps")
            for c in range(3):
                nc.tensor.matmul(
                    vps, anT[:, c, 128 * t:128 * (t + 1)], wv_sb[:, c, :],
                    start=(c == 0), stop=(c == 2),
                )
            nc.gpsimd.memset(v_aug[t], 0.0)
            nc.gpsimd.memset(v_aug[t][:, :, 0:1], 1.0)
            nc.vector.tensor_copy(
                v_aug[t][:, :, 1:CH + 1],
                vps.rearrange("p (h c) -> p h c", h=H),
            )

        def _piece_qg(j):
            # qk scale folded into Wq on the host
            qps = psp.tile([128, NQ], f32, tag="pps", name="qps")
            for c in range(3):
                nc.tensor.matmul(
                    qps, wq_sb[:, c, 128 * j:128 * (j + 1)], anTq[:, c, :],
                    start=(c == 0), stop=(c == 2),
                )
            nc.scalar.copy(qTt[j], qps)
            gps = psp.tile([128, NQ], f32, tag="pps", name="gps")
            for c in range(3):
                nc.tensor.matmul(
                    gps, wg_sb[:, c, 128 * j:128 * (j + 1)], anTq[:, c, :],
                    start=(c == 0), stop=(c == 2),
                )
            # sigmoid via the Exp table: g = 1/(1 + exp(-x - bg)), the -bg
            # arriving through the activation's per-partition bias port
            ge = sbpool.tile([128, NQ], f32, tag="ge", name=f"ge{j}")
            nc.scalar.activation(ge, gps, AF.Exp, scale=-1.0,
                                 bias=bgc[:, j:j + 1])
            gd = sbpool.tile([128, NQ], f32, tag="gd", name=f"gd{j}")
            nc.vector.tensor_scalar(gd, ge, 1.0, None, OP.add)
            with nc.allow_low_precision(reason="bf16 gating weights"):
                nc.vector.reciprocal(gT[:, j, :], gd)

        # ------------- phase C emitters -------------
        pt_t = {}

        def _scores(h, kg, add_on_pe=False):
            cn, j = h // 4, h % 4
            jb = 32 * j
            sc = scps.tile([128, KG, NQ], f32, tag="sc")
            bias = bias_sb[:, KG * kg:KG * (kg + 1), :, :, h]
            for ks in range(KG):
                kt = KG * kg + ks
                nc.tensor.matmul(
                    sc[:, ks, :],
                    kTt[cn][jb:jb + CHP, 128 * kt:128 * (kt + 1)],
                    qTt[cn][jb:jb + CHP, :],
                    start=(ks == 0), stop=(not add_on_pe and ks == KG - 1),
                    tile_position=(jb, 0), skip_group_check=True,
                )
            # pair+mask bias: identity matmul (PE) or strided add (DVE) —
            # the tail alternates so neither engine chains every head
            if add_on_pe:
                nc.tensor.matmul(
                    sc.rearrange("p a b -> p (a b)"), sb_id,
                    bias.rearrange("p t g q -> p (t g q)"),
                    start=False, stop=True, skip_group_check=True,
                )
            else:
                nc.vector.tensor_tensor(
                    sc, sc, bias.rearrange("p t g q -> p t (g q)"), OP.add)
            pool = pexp0 if kg == 0 else pexp
            p_t = pool.tile([128, KG, NQ], b16, tag="pt", name=f"pt{h}_{kg}")
            nc.scalar.activation(p_t, sc, AF.Exp)
            pt_t[(h, kg)] = p_t

        # ------------- emission schedule -------------
        pieces = (
            [lambda j=j: _piece_qg(j) for j in range(4)]
            + [lambda j=j, h=h: _piece_k(j, h) for h in range(2) for j in range(4)]
            + [lambda t=t: _piece_v(t) for t in range(KT)]
        )
        PC0 = 10                  # first phase-A piece slot
        SC0 = 20                  # first kg0 scores slot
        for g in range(4):        # prefetch 16 chunks of z
            _zdma(g)
        for chk in range(NCHUNK):
            if chk % DG == 0 and chk // DG + 4 < NCHUNK // DG:
                _zdma(chk // DG + 4)
            if chk % 2 == 1 and 3 + chk // 2 < (BLOBW + BP - 1) // BP:
                _blob_dma(3 + chk // 2)
            _chunk_mm(chk)
            if chk >= 3 and chk % 2 == 1:
                _pair_tail((chk - 3) // 2)
            s = chk - PC0
            if 0 <= s < len(pieces):
                pieces[s]()
            elif s == len(pieces):
                a_stack.close()
            if SC0 <= chk < SC0 + H:
                _scores(chk - SC0, 0)
        _pair_tail(NCHUNK // 2 - 1)
        b_stack.close()

        # ------------- post-loop: rest of kg0, then kg1 (lag-2) ------
        with (
            tc.tile_pool(name="otps", bufs=2, space="PSUM") as otps,
            tc.tile_pool(name="rbps", bufs=1, space="PSUM") as rbps,
            tc.tile_pool(name="psfin", bufs=1, space="PSUM") as psf,
            tc.tile_pool(name="tailp", bufs=2) as tailp,
        ):
            oT_t = {}
            goT = [const.tile([128, NQ], b16, name=f"goT{c}") for c in range(4)]
            ops = psf.tile([NQ, C], f32)

            def _pv6(h):
                cn, j = h // 4, h % 4
                jb = 32 * j
                if j == 0 and cn not in oT_t:
                    oT_t[cn] = otps.tile(
                        [128, NQ], f32, tag="oT", name=f"oT{cn}")
                for kg in range(2):
                    p_t = pt_t.pop((h, kg))
                    for ks in range(KG):
                        kt = KG * kg + ks
                        nc.tensor.matmul(
                            oT_t[cn][jb:jb + CHP, :], v_aug[kt][:, h, :],
                            p_t[:, ks, :],
                            start=(kt == 0), stop=(kt == KT - 1),
                            tile_position=(0, jb), skip_group_check=True,
                        )

            def _cn_tail(cn):
                oT = oT_t.pop(cn)
                rc = tailp.tile([128, NQ], b16, tag="rc", name=f"rc{cn}")
                with nc.allow_low_precision(reason="bf16 denominators"):
                    nc.vector.reciprocal(rc, oT)
                rb = rbps.tile([128, NQ], f32, tag="rb")
                for j in range(4):
                    jb = 32 * j
                    nc.tensor.matmul(
                        rb[jb:jb + CHP, :], ones_b[jb:jb + 1, :],
                        rc[jb:jb + 1, :],
                        tile_position=(jb, jb), skip_group_check=True,
                    )
                tmp = tailp.tile([128, NQ], f32, tag="tmp")
                nc.vector.tensor_tensor(tmp, oT, gT[:, cn, :], OP.mult)
                nc.vector.tensor_tensor(goT[cn], tmp, rb, OP.mult)
                nc.tensor.matmul(
                    ops, goT[cn], wo_sb[:, cn, :], start=(cn == 0),
                    stop=(cn == 3), skip_group_check=True,
                )

            for h in range(NCHUNK - SC0, H):
                _scores(h, 0)
            for h in range(H):
                if h >= 2:
                    _pv6(h - 2)
                _scores(h, 1)
                if h >= 2 and (h - 2) % 4 == 3:
                    _cn_tail((h - 2) // 4)
            _pv6(H - 2)
            _pv6(H - 1)
            _cn_tail(3)

            out_sb = tailp.tile([NQ, C], f32, tag="outsb")
            nc.vector.tensor_tensor(out_sb, ops, bo_b[0:NQ, :], OP.add)
            nc.sync.dma_start(out_d[:, :], out_sb)

    nc.compile()
    return nc


def _get_program():
    if "nc" not in _CACHE:
        _CACHE["nc"] = _build_program()
    return _CACHE["nc"]


def _pad_heads_cols(w, off):
    out = np.zeros((C, H, CHP), np.float32)
    out[:, :, off:off + CH] = np.asarray(w, np.float32).reshape(C, H, CH)
    return out.reshape(C, HP)


def _host_inputs(inputs):
    a = np.asarray(inputs["a"], np.float32)
    z = np.asarray(inputs["z"], np.float32)
    mask = np.asarray(inputs["mask"], np.float32)
    Wz = np.asarray(inputs["Wz"], np.float32)
    Wo = np.asarray(inputs["Wo"], np.float32)
    bg = np.asarray(inputs["bg"], np.float32)
    bo = np.asarray(inputs["bo"], np.float32)
    lnzw = np.asarray(inputs["ln_z_w"], np.float32)
    lnaw = np.asarray(inputs["ln_a_w"], np.float32)
    lnab = np.asarray(inputs["ln_a_b"], np.float32)

    # LN(a) folded on the host (elementwise only)
    mu = a.mean(axis=-1, keepdims=True)
    var = a.var(axis=-1, keepdims=True)
    an = ((a - mu) / np.sqrt(var + EPS) * lnaw + lnab)[0]   # [N, C]

    qscale = float(CH) ** -0.5
    Wq = qscale * np.asarray(inputs["Wq"], np.float32)
    Wk = np.asarray(inputs["Wk"], np.float32)
    Wg = np.asarray(inputs["Wg"], np.float32)
    Wv = np.asarray(inputs["Wv"], np.float32)

    wo_p = np.zeros((H, CHP, C), np.float32)
    wo_p[:, 1:CH + 1, :] = Wo.reshape(H, CH, C)
    bg_p = np.zeros((H, CHP), np.float32)
    bg_p[:, 1:CH + 1] = bg.reshape(H, CH)

    blob = np.zeros((128, BLOBW), np.float32)

    def _put3(nm, w):        # w: [384, width] -> [128, 3*width]
        o, tot = _BLOB[nm]
        width = tot // 3
        blob[:, o:o + tot] = w.reshape(3, 128, width).transpose(1, 0, 2).reshape(
            128, tot)

    def _put4(nm, w, width):  # w: [<=512, width] -> [128, 4*width]
        o, tot = _BLOB[nm]
        wp = np.zeros((4 * 128, width), np.float32)
        wp[:w.shape[0]] = w
        blob[:, o:o + tot] = wp.reshape(4, 128, width).transpose(1, 0, 2).reshape(
            128, tot)

    _put4("wq", _pad_heads_cols(Wq, 0), HP)
    _put4("wk", _pad_heads_cols(Wk, 0), HP)
    _put4("wg", _pad_heads_cols(Wg, 1), HP)
    _put3("wv", Wv)
    _put4("wo", wo_p.reshape(HP, C), C)
    _put3("anT", an.T.copy())            # anT[128c+p, t] -> [128, 3, 768]

    # centered fp8 half-stationaries [CZ, 2, 32]: [wza|0] and [0|wza]
    wzp = lnzw[:, None] * Wz
    wza_c = wzp - wzp.sum(axis=0, keepdims=True) / CZ
    wza = np.zeros((CZ, 2, 32), np.float32)
    wza[:, 0, 0:H] = wza_c
    wza[:, 1, H:2 * H] = wza_c

    bob = np.zeros((128, C + KT + 4), np.float32)
    bob[:, 0:C] = bo[None, :]
    bob[:, C:C + KT] = mask[0].reshape(KT, 128).T
    bob[:, C + KT:] = -bg_p.reshape(4, 128).T

    shared = {
        "blob_base": blob,
        "wza": wza.astype(float8_e4m3),
        "bob": bob,
        "ident": np.eye(128, dtype=bfloat16),
    }

    # z' = z * rstd, fp8, chunked kt-major for DoubleRow
    zf = z[0]
    zr = (zf * (1.0 / np.sqrt(zf.var(axis=-1) + EPS))[..., None]).astype(
        float8_e4m3)                      # [N(q), N(k), CZ]

    in_maps = []
    for core in range(NCORES):
        qs = slice(NQ * core, NQ * (core + 1))
        # [96, 768, 128] -> [CZ, KT, NQG, QG, 128k] -> [CZ, NCHUNK*2048]
        t = zr[qs].transpose(2, 1, 0).reshape(CZ, KT, 128, NQG, QG)
        t = t.transpose(0, 1, 3, 4, 2)
        zt = np.ascontiguousarray(t).reshape(CZ, NCHUNK * 2048)
        bl = blob.copy()
        o, tot = _BLOB["anTq"]
        bl[:, o:o + tot] = an[qs][PERM96].T.reshape(3, 128, NQ).transpose(
            1, 0, 2).reshape(128, tot)
        m = {k: v for k, v in shared.items() if k != "blob_base"}
        m["blob"] = bl.astype(bfloat16)
        m["zt"] = zt
        in_maps.append(m)
    return in_maps


def _run(inputs, trace=False):
    from concourse.bass_utils import run_bass_kernel_spmd

    nc = _get_program()
    in_maps = _host_inputs(inputs)
    res = run_bass_kernel_spmd(
        nc, in_maps, core_ids=list(range(NCORES)), trace=trace
    )
    rows = [res.results[i]["out"][PERM96] for i in range(NCORES)]
    out = np.concatenate(rows, axis=0).reshape(B, N, C).astype(np.float32)
    return out, res


def kernel(**inputs):
    out, _ = _run(inputs, trace=False)
    return out
